# revision 46
# baseline (speedup 1.0000x reference)
"""Trainium2 Bass kernel for nn_Encoder_61753039782402 (HD-computing encoder).

Math: out[b,d] = sign( sum_f parity( sum_t L[q(b,t,f), d-t] + sum_t id[f, d-t] ) - 20.5 )
where q(b,t,f) = trunc(16*x[b,t,f] - 1) wrapped mod 16 (x==0 -> 15).

Telescoped cumulative-mask formulation. Since q = floor(16x)-1 (with the
x in (0,1/16) and x==0 specials), the one-hot masks telescope into cumulative
thresholds g_k = [x >= k/16], k=2..15, contracted against signed delta bands
Delta_k = L[k-1]-L[k-2] (values in {-1,0,1}, exact in fp8e4m3):

  S = (window sum of L0) + S_id + sum_k g_k (*) Delta_k + z (*) (L15-L0)

No floor chain; masks are single compares on raw x, split across engines:
  - DVE: z = [x==0] plus 7 is_ge compares
  - GPSIMD: 3 is_ge compares
  - ACT: 4 Sign-activation masks h_k = sign(16x - k + 2^-21) in {-1,+1}; the
    +-1-vs-0/1 offset is folded into the constant id pass (those bands are
    pre-scaled by 0.5 host-side, id table gets +(L4-L0)/2). The 2^-21
    tie-break makes the x == k/16 boundary exact without relying on sign(0)
    (argument is never zero; bias 2^-21-k is exactly representable for k<8).
    A dummy Sign op at program start pre-loads the ACT function table so the
    1.3us table load happens while waiting for x.

Channels are numbered so DoubleRow pairs become ready in ascending order
(pair = one DVE mask + one ACT/Pool mask finishing at the same time), and
the band table is split into 3 DMAs so early pairs' stationary tiles land
(and their +900ns completion sems fire) before late ones.

The id/L0 constant term goes through one DoubleRow pass per chunk with a
host-baked triangular mask against idp = id + L0/2 + L4/2 (exact in fp8).
Parity+reduce tail: PSUM->i16 converts split across DVE and ACT, packed-i16
bitwise-and (DVE 4x mode), one grouped reduce over both chunks, and a
single-op threshold to {0,2} (the constant -1 relabel to +-1 happens during
host-side assembly). A dummy matmul at program start ramps the PE p-state;
three PE warmup passes keep later matmuls at the fast cycle. Single output
DMA via SP's HWDGE (lowest trigger+DGE latency).

Host-side prep is layout/dtype/table work only (shift-windows, deltas and
halvings of the 0/1 tables, fp8 casts, replication); all x-dependent compute
and all window summation happens on device.
"""

from contextlib import ExitStack

import numpy as np
import ml_dtypes

import concourse.bass as bass
import concourse.bacc as bacc
import concourse.mybir as mybir
import concourse.tile as tile
from concourse.bass_utils import run_bass_kernel_spmd

B, T, F, Q, D = 8, 128, 40, 16, 2048
NCORE = 8
DS = D // NCORE  # 256 output columns per core
BF = B * F       # 320
f32, bf16, i32 = mybir.dt.float32, mybir.dt.bfloat16, mybir.dt.int32
i16 = mybir.dt.int16
f8 = mybir.dt.float8e4
AL = mybir.AluOpType
AF = mybir.ActivationFunctionType
EPS = 2.0 ** -21

# channel layout: pairs (2i, 2i+1) are DoubleRow partners, numbered by
# expected mask readiness. ch0 = z, ch1 = spare(zero band).
DVE_CH2K = {2: 6, 4: 7, 6: 8, 8: 9, 10: 10, 12: 11, 14: 12}
ACT_CH2K = {3: 2, 7: 3, 11: 4, 15: 5}
POOL_CH2K = {5: 13, 9: 14, 13: 15}
Z_CH, SPARE_CH = 0, 1
# band DMA split by pair groups (channel ranges), in arrival order; the
# first two ride SP's HWDGE, the tiny last group rides Pool's SWDGE so its
# (+900ns) completion sem gates only the final pair's two passes
BAND_SPLITS = [(0, 6), (6, 12), (12, 16)]

N_PE_WARMUP = 3


def emit_pre_tile(nc, out_d):
    """Raw fin tensor allocated outside the tile pools (address fixed at
    emission); the out DMA itself is a plain HWDGE dma_start in-tile."""
    fin_t = nc.alloc_sbuf_tensor("fin_raw", [128, 1, 1, 16], f32)
    return out_d, fin_t


def emit_kernel(nc, tc, ctx, xt_d, bnd_ds, cst_d, pre):
    sb = ctx.enter_context(tc.tile_pool(name="sb", bufs=1))
    psp = ctx.enter_context(tc.tile_pool(name="psp", bufs=1, space=bass.MemorySpace.PSUM))
    DR = mybir.MatmulPerfMode.DoubleRow
    out_d, fin_t = pre
    fin = fin_t.ap()

    # ---- input DMAs ------------------------------------------------------
    # HWDGE triggers on SP in program order: x first (critical), then band
    # groups in pair order. consts ride Pool's SWDGE (engine idle early).
    xt = sb.tile([T, B, F], f32, tag="xt")
    nc.sync.dma_start(out=xt[:], in_=xt_d)
    xt2 = xt[:].rearrange("u b f -> u (b f)")  # [128, 320]

    sla = sb.tile([128, 2, Q, 128], f8, tag="sla")  # [u, bank, ch, d']
    for (c0, c1), bd in zip(BAND_SPLITS, bnd_ds):
        nc.sync.dma_start(out=sla[:, :, c0:c1, :].rearrange("p m c d -> p m (c d)"),
                          in_=bd)

    cst = sb.tile([128, 1216], f8, tag="cst")
    nc.gpsimd.dma_start(out=cst[:], in_=cst_d)
    triv = cst[:, 0:256].rearrange("p (j m) -> p j m", j=2)       # [128, 2, 128]
    idrv = cst[:, 256:1216].rearrange("p (j bf) -> p j bf", j=3)  # [128, 3, 320]

    # ---- early constant setup (engines idle until x lands) ---------------
    bia = sb.tile([128, 8], f32, tag="bia")
    for i, k in enumerate(ACT_CH2K.values()):
        nc.vector.memset(bia[:, i:i + 1], EPS - float(k))
    nc.vector.memset(bia[:, 5:6], 0.0)

    # pre-load the ACT Sign function table while waiting for x
    scr = sb.tile([128, 1], f32, tag="scr")
    nc.scalar.activation(out=scr[:], in_=bia[:, 5:6], func=AF.Sign,
                         bias=bia[:, 5:6], scale=1.0)

    oha = sb.tile([T, Q, BF], f8, tag="oha")
    nc.vector.memset(oha[:, SPARE_CH, :], 0.0)

    dw = sb.tile([128, 64], f8, tag="dw")
    nc.vector.memset(dw[:], 0.0)
    psD = psp.tile([64, 64], f32, tag="psD")
    for _ in range(N_PE_WARMUP):
        nc.tensor.matmul(psD[:], dw[:], dw[:], start=True, stop=True)

    # ---- masks -----------------------------------------------------------
    nc.vector.tensor_single_scalar(out=oha[:, Z_CH, :], in_=xt2, scalar=0.0,
                                   op=AL.is_equal)
    for ch, k in DVE_CH2K.items():
        nc.vector.tensor_single_scalar(out=oha[:, ch, :], in_=xt2,
                                       scalar=float(k) / 16.0, op=AL.is_ge)
    for ch, k in POOL_CH2K.items():
        nc.gpsimd.tensor_single_scalar(out=oha[:, ch, :], in_=xt2,
                                       scalar=float(k) / 16.0, op=AL.is_ge)
    for i, (ch, k) in enumerate(ACT_CH2K.items()):
        nc.scalar.activation(out=oha[:, ch, :], in_=xt2, func=AF.Sign,
                             bias=bia[:, i:i + 1], scale=16.0)

    # ---- matmul chains ---------------------------------------------------
    pA = psp.tile([128, BF], f32, tag="accA")
    pB = psp.tile([128, BF], f32, tag="accB")
    nc.tensor.matmul(pA[:], triv, idrv[:, 0:2], start=True, stop=False, perf_mode=DR)
    nc.tensor.matmul(pB[:], triv, idrv[:, 1:3], start=True, stop=False, perf_mode=DR)
    for ci in range(8):
        ca, cb = 2 * ci, 2 * ci + 1
        last = ci == 7
        nc.tensor.matmul(pA[:], sla[:, 0, ca:cb + 1, :], oha[:, ca:cb + 1, :],
                         start=False, stop=last, perf_mode=DR)
        nc.tensor.matmul(pB[:], sla[:, 1, ca:cb + 1, :], oha[:, ca:cb + 1, :],
                         start=False, stop=last, perf_mode=DR)

    # ---- parity + grouped reduce + threshold -----------------------------
    # i16 throughout: 2-byte packed operands unlock DVE 2x/4x modes; values
    # fit (S <= 256, group sums <= 40)
    si = sb.tile([128, 2, BF], i16, tag="si")
    nc.vector.tensor_copy(out=si[:, 0], in_=pA[:])
    nc.scalar.activation(out=si[:, 1], in_=pB[:], func=AF.Copy, bias=0.0, scale=1.0)
    par = sb.tile([128, 2, B, F], i16, tag="par")
    nc.vector.tensor_single_scalar(out=par[:, 0].rearrange("p b f -> p (b f)"),
                                   in_=si[:, 0], scalar=1, op=AL.bitwise_and)
    nc.vector.tensor_single_scalar(out=par[:, 1].rearrange("p b f -> p (b f)"),
                                   in_=si[:, 1], scalar=1, op=AL.bitwise_and)
    red = sb.tile([128, 2, B], i16, tag="red")
    with nc.allow_low_precision(reason="exact small-int accumulation (<=40)"):
        nc.vector.tensor_reduce(out=red[:], in_=par[:],
                                axis=mybir.AxisListType.X, op=AL.add)
    # threshold on DVE, one op: device classifies to {0, 2}; the constant
    # -1 relabel to {-1, +1} happens during host-side unshard/assembly
    nc.vector.tensor_scalar(out=fin[:, 0, 0, :],
                            in0=red[:].rearrange("p m b -> p (m b)"),
                            scalar1=20, scalar2=2.0, op0=AL.is_gt, op1=AL.mult)
    nc.sync.dma_start(out=out_d, in_=fin)


def build_nc():
    nc = bacc.Bacc("TRN2", target_bir_lowering=False, debug=False)
    # Startup-barrier surgery. The Bass-constructor barrier only orders the
    # const-AP registration memsets (which nothing in this kernel reads, and
    # which are moved to DVE where they are free). SP's only pre-compute work
    # is firing the input DMA triggers, so release SP from the barrier: drop
    # its waits and its release-decrement, and lower Pool's release-add from
    # 4 to 3 so the gather/release accounting still balances for the other
    # engines (final sem state unchanged; no negative-sem transitions).
    _ms_n = 0
    for bb in nc.m.functions[0].blocks:
        for ins in bb.instructions:
            si = ins.sync_info
            if type(ins).__name__ == "InstMemset" and ins.engine == mybir.EngineType.Pool:
                ins.engine = mybir.EngineType.DVE
            if not si:
                continue
            if any("barrier" in str(w) for w in si.on_wait):
                si.on_wait = [w for w in si.on_wait if "barrier" not in str(w)]
            if any("barrier" in str(u) for u in si.on_update):
                si.on_update = [u for u in si.on_update if "barrier" not in str(u)]
        break
    xt_d = nc.dram_tensor("xt", [T, B, F], f32, kind="ExternalInput")
    bnd_ds = [nc.dram_tensor(f"bnd{i}", [128, 2 * (c1 - c0) * 128], f8,
                             kind="ExternalInput")
              for i, (c0, c1) in enumerate(BAND_SPLITS)]
    cst_d = nc.dram_tensor("cst", [128, 1216], f8, kind="ExternalInput")
    out_d = nc.dram_tensor("out", [1, 128, 1, 16], f32, kind="ExternalOutput")
    pre = emit_pre_tile(nc, out_d[:])
    with tile.TileContext(nc) as tc:
        with ExitStack() as ctx:
            emit_kernel(nc, tc, ctx, xt_d[:], [bd[:] for bd in bnd_ds],
                        cst_d[:], pre)
    nc.compile()
    return nc


def make_in_maps(x, level_hvs, id_hvs):
    x = np.asarray(x, dtype=np.float32)
    L = np.asarray(level_hvs, dtype=np.int32)
    ID = np.asarray(id_hvs, dtype=np.int32)
    # time-reverse + transpose to [T, B, F] (so band indices are u + d')
    xt = np.ascontiguousarray(x[:, ::-1, :].transpose(1, 0, 2))

    # signed delta band tables per channel
    Btab = np.zeros((Q, D), np.float32)
    for ch, k in {**DVE_CH2K, **ACT_CH2K, **POOL_CH2K}.items():
        Btab[ch] = (L[k - 1] - L[k - 2]).astype(np.float32)
        if ch in ACT_CH2K:
            Btab[ch] *= 0.5  # +-1 sign-masks contribute h*Delta/2
    Btab[Z_CH] = (L[15] - L[0]).astype(np.float32)
    # Btab[SPARE_CH] stays 0
    Btab2 = np.ascontiguousarray(
        np.concatenate([Btab, Btab], axis=1)).astype(ml_dtypes.float8_e4m3)

    # constant id pass table: id + L0 + sum_{k in ACT} Delta_k/2 = id+L0/2+L4/2
    ks = sorted(ACT_CH2K.values())
    assert ks == list(range(ks[0], ks[0] + len(ks)))
    idp = ID.astype(np.float32) + 0.5 * L[ks[0] - 2] + 0.5 * L[ks[-1] - 1]
    idp2 = np.concatenate([idp, idp], axis=1)  # [40, 4096]

    m = np.arange(128)
    tri = np.zeros((128, 2, 128), np.float32)
    tri[:, 0, :] = (m[None, :] < m[:, None])
    tri[:, 1, :] = (m[None, :] >= m[:, None])
    tri8 = tri.reshape(128, 256).astype(ml_dtypes.float8_e4m3)

    flat = Btab2.reshape(-1)
    in_maps = []
    for c in range(NCORE):
        d0 = c * DS
        s = (d0 - 127) % D
        band = np.lib.stride_tricks.as_strided(
            flat[s:], shape=(128, Q, DS), strides=(1, 2 * D, 1))
        # [u, bank, ch, d']
        bnd = np.ascontiguousarray(
            np.asarray(band).reshape(128, Q, 2, 128).transpose(0, 2, 1, 3))
        core_map = {"xt": xt, }
        for i, (c0, c1) in enumerate(BAND_SPLITS):
            core_map[f"bnd{i}"] = np.ascontiguousarray(
                bnd[:, :, c0:c1, :]).reshape(128, 2 * (c1 - c0) * 128)
        s2 = (d0 - 128) % D
        idt_c = idp2[:, s2:s2 + 384].T                         # [384, 40]
        idt_full = np.broadcast_to(idt_c[:, None, :], (384, B, F)).reshape(384, BF)
        idt_r = np.ascontiguousarray(
            idt_full.reshape(3, 128, BF).transpose(1, 0, 2)).reshape(128, 3 * BF)
        core_map["cst"] = np.ascontiguousarray(np.concatenate(
            [tri8, idt_r.astype(ml_dtypes.float8_e4m3)], axis=1))
        in_maps.append(core_map)
    return in_maps


_NC_CACHE = {}


def kernel(x, level_hvs, id_hvs):
    if "nc" not in _NC_CACHE:
        _NC_CACHE["nc"] = build_nc()
    nc = _NC_CACHE["nc"]
    in_maps = make_in_maps(x, level_hvs, id_hvs)
    res = run_bass_kernel_spmd(nc, in_maps, list(range(NCORE)))
    full = np.empty((B, D), dtype=np.float32)
    for c in range(NCORE):
        o = np.asarray(res.results[c]["out"]).reshape(128, 2, B)  # [p, mc, b]
        full[:, c * DS:(c + 1) * DS] = o.transpose(2, 1, 0).reshape(B, DS) - 1.0
    return full


# revision 48
# speedup vs baseline: 1.0235x; 1.0235x over previous
"""Trainium2 Bass kernel for nn_Encoder_61753039782402 (HD-computing encoder).

Math: out[b,d] = sign( sum_f parity( sum_t L[q(b,t,f), d-t] + sum_t id[f, d-t] ) - 20.5 )
where q(b,t,f) = trunc(16*x[b,t,f] - 1) wrapped mod 16 (x==0 -> 15).

Telescoped cumulative-mask formulation. Since q = floor(16x)-1 (with the
x in (0,1/16) and x==0 specials), the one-hot masks telescope into cumulative
thresholds g_k = [x >= k/16], k=2..15, contracted against signed delta bands
Delta_k = L[k-1]-L[k-2] (values in {-1,0,1}, exact in fp8e4m3):

  S = (window sum of L0) + S_id + sum_k g_k (*) Delta_k + z (*) (L15-L0)

No floor chain; masks are single compares on raw x, split across engines:
  - DVE: z = [x==0] plus 7 is_ge compares
  - GPSIMD: 3 is_ge compares
  - ACT: 4 Sign-activation masks h_k = sign(16x - k + 2^-21) in {-1,+1}; the
    +-1-vs-0/1 offset is folded into the constant id pass (those bands are
    pre-scaled by 0.5 host-side, id table gets +(L4-L0)/2). The 2^-21
    tie-break makes the x == k/16 boundary exact without relying on sign(0)
    (argument is never zero; bias 2^-21-k is exactly representable for k<8).
    A dummy Sign op at program start pre-loads the ACT function table so the
    1.3us table load happens while waiting for x.

Channels are numbered so DoubleRow pairs become ready in ascending order
(pair = one DVE mask + one ACT/Pool mask finishing at the same time), and
the band table is split into 3 DMAs so early pairs' stationary tiles land
(and their +900ns completion sems fire) before late ones.

The id/L0 constant term goes through one DoubleRow pass per chunk with a
host-baked triangular mask against idp = id + L0/2 + L4/2 (exact in fp8).
Parity+reduce tail: PSUM->i16 converts split across DVE and ACT, packed-i16
bitwise-and (DVE 4x mode), one grouped reduce over both chunks, and a
single-op threshold to {0,2} (the constant -1 relabel to +-1 happens during
host-side assembly). A dummy matmul at program start ramps the PE p-state;
three PE warmup passes keep later matmuls at the fast cycle. Single output
DMA via SP's HWDGE (lowest trigger+DGE latency).

Host-side prep is layout/dtype/table work only (shift-windows, deltas and
halvings of the 0/1 tables, fp8 casts, replication); all x-dependent compute
and all window summation happens on device.
"""

from contextlib import ExitStack

import numpy as np
import ml_dtypes

import concourse.bass as bass
import concourse.bacc as bacc
import concourse.mybir as mybir
import concourse.tile as tile
from concourse.bass_utils import run_bass_kernel_spmd

B, T, F, Q, D = 8, 128, 40, 16, 2048
NCORE = 8
DS = D // NCORE  # 256 output columns per core
BF = B * F       # 320
f32, bf16, i32 = mybir.dt.float32, mybir.dt.bfloat16, mybir.dt.int32
i16 = mybir.dt.int16
f8 = mybir.dt.float8e4
AL = mybir.AluOpType
AF = mybir.ActivationFunctionType
EPS = 2.0 ** -21

# channel layout: pairs (2i, 2i+1) are DoubleRow partners, numbered by
# expected mask readiness. ch0 = z, ch1 = spare(zero band).
DVE_CH2K = {2: 6, 4: 7, 6: 8, 8: 9, 10: 10, 12: 11, 14: 12}
ACT_CH2K = {3: 2, 7: 3, 11: 4, 15: 5}
POOL_CH2K = {5: 13, 9: 14, 13: 15}
Z_CH, SPARE_CH = 0, 1
# band DMA split by pair groups (channel ranges), in arrival order; the
# first two ride SP's HWDGE, the tiny last group rides Pool's SWDGE so its
# (+900ns) completion sem gates only the final pair's two passes
BAND_SPLITS = [(0, 6), (6, 12), (12, 16)]

N_PE_WARMUP = 3


def emit_pre_tile(nc, out_d):
    """Raw fin tensor allocated outside the tile pools (address fixed at
    emission); the out DMA itself is a plain HWDGE dma_start in-tile."""
    fin_t = nc.alloc_sbuf_tensor("fin_raw", [128, 1, 1, 16], f32)
    return out_d, fin_t


def emit_kernel(nc, tc, ctx, xt_d, bnd_ds, cst_d, pre):
    sb = ctx.enter_context(tc.tile_pool(name="sb", bufs=1))
    psp = ctx.enter_context(tc.tile_pool(name="psp", bufs=1, space=bass.MemorySpace.PSUM))
    DR = mybir.MatmulPerfMode.DoubleRow
    out_d, fin_t = pre
    fin = fin_t.ap()

    # ---- input DMAs ------------------------------------------------------
    # HWDGE triggers on SP in program order: x first (critical), then band
    # groups in pair order. consts ride Pool's SWDGE (engine idle early).
    xt = sb.tile([T, B, F], f32, tag="xt")
    nc.sync.dma_start(out=xt[:], in_=xt_d)
    xt2 = xt[:].rearrange("u b f -> u (b f)")  # [128, 320]

    sla = sb.tile([128, 2, Q, 128], f8, tag="sla")  # [u, bank, ch, d']
    for (c0, c1), bd in zip(BAND_SPLITS, bnd_ds):
        nc.sync.dma_start(out=sla[:, :, c0:c1, :].rearrange("p m c d -> p m (c d)"),
                          in_=bd)

    cst = sb.tile([128, 1216], f8, tag="cst")
    nc.gpsimd.dma_start(out=cst[:], in_=cst_d)
    triv = cst[:, 0:256].rearrange("p (j m) -> p j m", j=2)       # [128, 2, 128]
    idrv = cst[:, 256:1216].rearrange("p (j bf) -> p j bf", j=3)  # [128, 3, 320]

    # ---- early constant setup (engines idle until x lands) ---------------
    bia = sb.tile([128, 8], f32, tag="bia")
    for i, k in enumerate(ACT_CH2K.values()):
        nc.vector.memset(bia[:, i:i + 1], EPS - float(k))
    nc.vector.memset(bia[:, 5:6], 0.0)

    # pre-load the ACT Sign function table while waiting for x
    scr = sb.tile([128, 1], f32, tag="scr")
    nc.scalar.activation(out=scr[:], in_=bia[:, 5:6], func=AF.Sign,
                         bias=bia[:, 5:6], scale=1.0)

    oha = sb.tile([T, Q, BF], f8, tag="oha")
    nc.vector.memset(oha[:, SPARE_CH, :], 0.0)

    dw = sb.tile([128, 64], f8, tag="dw")
    nc.vector.memset(dw[:], 0.0)
    psD = psp.tile([64, 64], f32, tag="psD")
    for _ in range(N_PE_WARMUP):
        nc.tensor.matmul(psD[:], dw[:], dw[:], start=True, stop=True)

    # ---- masks -----------------------------------------------------------
    nc.vector.tensor_single_scalar(out=oha[:, Z_CH, :], in_=xt2, scalar=0.0,
                                   op=AL.is_equal)
    for ch, k in DVE_CH2K.items():
        nc.vector.tensor_single_scalar(out=oha[:, ch, :], in_=xt2,
                                       scalar=float(k) / 16.0, op=AL.is_ge)
    for ch, k in POOL_CH2K.items():
        nc.gpsimd.tensor_single_scalar(out=oha[:, ch, :], in_=xt2,
                                       scalar=float(k) / 16.0, op=AL.is_ge)
    for i, (ch, k) in enumerate(ACT_CH2K.items()):
        nc.scalar.activation(out=oha[:, ch, :], in_=xt2, func=AF.Sign,
                             bias=bia[:, i:i + 1], scale=16.0)

    # ---- matmul chains ---------------------------------------------------
    pA = psp.tile([128, BF], f32, tag="accA")
    pB = psp.tile([128, BF], f32, tag="accB")
    nc.tensor.matmul(pA[:], triv, idrv[:, 0:2], start=True, stop=False, perf_mode=DR)
    nc.tensor.matmul(pB[:], triv, idrv[:, 1:3], start=True, stop=False, perf_mode=DR)
    for ci in range(8):
        ca, cb = 2 * ci, 2 * ci + 1
        last = ci == 7
        nc.tensor.matmul(pA[:], sla[:, 0, ca:cb + 1, :], oha[:, ca:cb + 1, :],
                         start=False, stop=last, perf_mode=DR)
        nc.tensor.matmul(pB[:], sla[:, 1, ca:cb + 1, :], oha[:, ca:cb + 1, :],
                         start=False, stop=last, perf_mode=DR)

    # ---- parity + grouped reduce + threshold -----------------------------
    # i16 throughout: 2-byte packed operands unlock DVE 2x/4x modes; values
    # fit (S <= 256, group sums <= 40)
    si = sb.tile([128, 2, BF], i16, tag="si")
    nc.vector.tensor_copy(out=si[:, 0], in_=pA[:])
    nc.scalar.activation(out=si[:, 1], in_=pB[:], func=AF.Copy, bias=0.0, scale=1.0)
    par = sb.tile([128, 2, B, F], i16, tag="par")
    nc.vector.tensor_single_scalar(out=par[:, 0].rearrange("p b f -> p (b f)"),
                                   in_=si[:, 0], scalar=1, op=AL.bitwise_and)
    nc.vector.tensor_single_scalar(out=par[:, 1].rearrange("p b f -> p (b f)"),
                                   in_=si[:, 1], scalar=1, op=AL.bitwise_and)
    red = sb.tile([128, 2, B], i16, tag="red")
    with nc.allow_low_precision(reason="exact small-int accumulation (<=40)"):
        nc.vector.tensor_reduce(out=red[:], in_=par[:],
                                axis=mybir.AxisListType.X, op=AL.add)
    # threshold on DVE, one op: device classifies to {0, 2}; the constant
    # -1 relabel to {-1, +1} happens during host-side unshard/assembly
    nc.vector.tensor_scalar(out=fin[:, 0, 0, :],
                            in0=red[:].rearrange("p m b -> p (m b)"),
                            scalar1=20, scalar2=2.0, op0=AL.is_gt, op1=AL.mult)
    nc.sync.dma_start(out=out_d, in_=fin)


def build_nc():
    nc = bacc.Bacc("TRN2", target_bir_lowering=False, debug=False)
    # Startup-barrier surgery. The Bass-constructor barrier only orders the
    # const-AP registration memsets (which nothing in this kernel reads, and
    # which are moved to DVE where they are free). SP's only pre-compute work
    # is firing the input DMA triggers, so release SP from the barrier: drop
    # its waits and its release-decrement, and lower Pool's release-add from
    # 4 to 3 so the gather/release accounting still balances for the other
    # engines (final sem state unchanged; no negative-sem transitions).
    _ms_n = 0
    for bb in nc.m.functions[0].blocks:
        for ins in bb.instructions:
            si = ins.sync_info
            if type(ins).__name__ == "InstMemset" and ins.engine == mybir.EngineType.Pool:
                ins.engine = mybir.EngineType.DVE
            if not si:
                continue
            if any("barrier" in str(w) for w in si.on_wait):
                si.on_wait = [w for w in si.on_wait if "barrier" not in str(w)]
            if any("barrier" in str(u) for u in si.on_update):
                si.on_update = [u for u in si.on_update if "barrier" not in str(u)]
        break
    xt_d = nc.dram_tensor("xt", [T, B, F], f32, kind="ExternalInput")
    bnd_ds = [nc.dram_tensor(f"bnd{i}", [128, 2 * (c1 - c0) * 128], f8,
                             kind="ExternalInput")
              for i, (c0, c1) in enumerate(BAND_SPLITS)]
    cst_d = nc.dram_tensor("cst", [128, 1216], f8, kind="ExternalInput")
    out_d = nc.dram_tensor("out", [1, 128, 1, 16], f32, kind="ExternalOutput")
    pre = emit_pre_tile(nc, out_d[:])
    with tile.TileContext(nc) as tc:
        with ExitStack() as ctx:
            emit_kernel(nc, tc, ctx, xt_d[:], [bd[:] for bd in bnd_ds],
                        cst_d[:], pre)
    # Closing-barrier surgery: ACT/PE/DVE have no DMA-ring duties and their
    # results are all consumed via Tile data-flow sems, so they may pass the
    # closing barriers and halt without waiting. They keep their gather
    # increments (Pool still waits for all four engines -> its DGE drain
    # stays ordered after SP's out-DMA wait); their release decrements are
    # removed and Pool's release add drops 4 -> 1 so only SP's handshake
    # remains and no semaphore goes negative.
    EARLY = (mybir.EngineType.Activation, mybir.EngineType.PE, mybir.EngineType.DVE)
    seen = {}
    for bb in list(nc.m.functions[0].blocks)[1:]:
        for ins in bb.instructions:
            si = ins.sync_info
            if not si:
                continue
            is_bar = (any("barrier" in str(w) for w in si.on_wait)
                      or any("barrier" in str(u) for u in si.on_update))
            if not is_bar:
                continue
            n = seen.get(ins.engine, 0)
            seen[ins.engine] = n + 1
            if n >= 2:
                # second closing barrier: redundant once the first orders
                # Pool's drain after SP -> strip entirely for all engines
                si.on_wait = [w for w in si.on_wait if "barrier" not in str(w)]
                si.on_update = [u for u in si.on_update if "barrier" not in str(u)]
                continue
            if ins.engine in EARLY:
                si.on_wait = [w for w in si.on_wait if "barrier" not in str(w)]
                si.on_update = [u for u in si.on_update
                                if not ("release" in str(u) and "sem-dec" in str(u))]
            for u in si.on_update:
                if ("release" in str(u) and "sem-add-imm" in str(u)
                        and u.update_value == 4):
                    u.update_value = 1
    nc.compile()
    return nc


def make_in_maps(x, level_hvs, id_hvs):
    x = np.asarray(x, dtype=np.float32)
    L = np.asarray(level_hvs, dtype=np.int32)
    ID = np.asarray(id_hvs, dtype=np.int32)
    # time-reverse + transpose to [T, B, F] (so band indices are u + d')
    xt = np.ascontiguousarray(x[:, ::-1, :].transpose(1, 0, 2))

    # signed delta band tables per channel
    Btab = np.zeros((Q, D), np.float32)
    for ch, k in {**DVE_CH2K, **ACT_CH2K, **POOL_CH2K}.items():
        Btab[ch] = (L[k - 1] - L[k - 2]).astype(np.float32)
        if ch in ACT_CH2K:
            Btab[ch] *= 0.5  # +-1 sign-masks contribute h*Delta/2
    Btab[Z_CH] = (L[15] - L[0]).astype(np.float32)
    # Btab[SPARE_CH] stays 0
    Btab2 = np.ascontiguousarray(
        np.concatenate([Btab, Btab], axis=1)).astype(ml_dtypes.float8_e4m3)

    # constant id pass table: id + L0 + sum_{k in ACT} Delta_k/2 = id+L0/2+L4/2
    ks = sorted(ACT_CH2K.values())
    assert ks == list(range(ks[0], ks[0] + len(ks)))
    idp = ID.astype(np.float32) + 0.5 * L[ks[0] - 2] + 0.5 * L[ks[-1] - 1]
    idp2 = np.concatenate([idp, idp], axis=1)  # [40, 4096]

    m = np.arange(128)
    tri = np.zeros((128, 2, 128), np.float32)
    tri[:, 0, :] = (m[None, :] < m[:, None])
    tri[:, 1, :] = (m[None, :] >= m[:, None])
    tri8 = tri.reshape(128, 256).astype(ml_dtypes.float8_e4m3)

    flat = Btab2.reshape(-1)
    in_maps = []
    for c in range(NCORE):
        d0 = c * DS
        s = (d0 - 127) % D
        band = np.lib.stride_tricks.as_strided(
            flat[s:], shape=(128, Q, DS), strides=(1, 2 * D, 1))
        # [u, bank, ch, d']
        bnd = np.ascontiguousarray(
            np.asarray(band).reshape(128, Q, 2, 128).transpose(0, 2, 1, 3))
        core_map = {"xt": xt, }
        for i, (c0, c1) in enumerate(BAND_SPLITS):
            core_map[f"bnd{i}"] = np.ascontiguousarray(
                bnd[:, :, c0:c1, :]).reshape(128, 2 * (c1 - c0) * 128)
        s2 = (d0 - 128) % D
        idt_c = idp2[:, s2:s2 + 384].T                         # [384, 40]
        idt_full = np.broadcast_to(idt_c[:, None, :], (384, B, F)).reshape(384, BF)
        idt_r = np.ascontiguousarray(
            idt_full.reshape(3, 128, BF).transpose(1, 0, 2)).reshape(128, 3 * BF)
        core_map["cst"] = np.ascontiguousarray(np.concatenate(
            [tri8, idt_r.astype(ml_dtypes.float8_e4m3)], axis=1))
        in_maps.append(core_map)
    return in_maps


_NC_CACHE = {}


def kernel(x, level_hvs, id_hvs):
    if "nc" not in _NC_CACHE:
        _NC_CACHE["nc"] = build_nc()
    nc = _NC_CACHE["nc"]
    in_maps = make_in_maps(x, level_hvs, id_hvs)
    res = run_bass_kernel_spmd(nc, in_maps, list(range(NCORE)))
    full = np.empty((B, D), dtype=np.float32)
    for c in range(NCORE):
        o = np.asarray(res.results[c]["out"]).reshape(128, 2, B)  # [p, mc, b]
        full[:, c * DS:(c + 1) * DS] = o.transpose(2, 1, 0).reshape(B, DS) - 1.0
    return full


# revision 49
# speedup vs baseline: 1.0401x; 1.0162x over previous
"""Trainium2 Bass kernel for nn_Encoder_61753039782402 (HD-computing encoder).

Math: out[b,d] = sign( sum_f parity( sum_t L[q(b,t,f), d-t] + sum_t id[f, d-t] ) - 20.5 )
where q(b,t,f) = trunc(16*x[b,t,f] - 1) wrapped mod 16 (x==0 -> 15).

Telescoped cumulative-mask formulation. Since q = floor(16x)-1 (with the
x in (0,1/16) and x==0 specials), the one-hot masks telescope into cumulative
thresholds g_k = [x >= k/16], k=2..15, contracted against signed delta bands
Delta_k = L[k-1]-L[k-2] (values in {-1,0,1}, exact in fp8e4m3):

  S = (window sum of L0) + S_id + sum_k g_k (*) Delta_k + z (*) (L15-L0)

No floor chain; masks are single compares on raw x, split across engines:
  - DVE: z = [x==0] plus 7 is_ge compares
  - GPSIMD: 3 is_ge compares
  - ACT: 4 Sign-activation masks h_k = sign(16x - k + 2^-21) in {-1,+1}; the
    +-1-vs-0/1 offset is folded into the constant id pass (those bands are
    pre-scaled by 0.5 host-side, id table gets +(L4-L0)/2). The 2^-21
    tie-break makes the x == k/16 boundary exact without relying on sign(0)
    (argument is never zero; bias 2^-21-k is exactly representable for k<8).
    A dummy Sign op at program start pre-loads the ACT function table so the
    1.3us table load happens while waiting for x.

Channels are numbered so DoubleRow pairs become ready in ascending order
(pair = one DVE mask + one ACT/Pool mask finishing at the same time), and
the band table is split into 3 DMAs so early pairs' stationary tiles land
(and their +900ns completion sems fire) before late ones.

The id/L0 constant term goes through one DoubleRow pass per chunk with a
host-baked triangular mask against idp = id + L0/2 + L4/2 (exact in fp8).
Parity+reduce tail: PSUM->i16 converts split across DVE and ACT, packed-i16
bitwise-and (DVE 4x mode), one grouped reduce over both chunks, and a
single-op threshold to {0,2} (the constant -1 relabel to +-1 happens during
host-side assembly). A dummy matmul at program start ramps the PE p-state;
three PE warmup passes keep later matmuls at the fast cycle. Single output
DMA via SP's HWDGE (lowest trigger+DGE latency).

Host-side prep is layout/dtype/table work only (shift-windows, deltas and
halvings of the 0/1 tables, fp8 casts, replication); all x-dependent compute
and all window summation happens on device.
"""

from contextlib import ExitStack

import numpy as np
import ml_dtypes

import concourse.bass as bass
import concourse.bacc as bacc
import concourse.mybir as mybir
import concourse.tile as tile
from concourse.bass_utils import run_bass_kernel_spmd

B, T, F, Q, D = 8, 128, 40, 16, 2048
NCORE = 8
DS = D // NCORE  # 256 output columns per core
BF = B * F       # 320
f32, bf16, i32 = mybir.dt.float32, mybir.dt.bfloat16, mybir.dt.int32
i16 = mybir.dt.int16
f8 = mybir.dt.float8e4
AL = mybir.AluOpType
AF = mybir.ActivationFunctionType
EPS = 2.0 ** -21

# channel layout: pairs (2i, 2i+1) are DoubleRow partners, numbered by
# expected mask readiness. ch0 = z, ch1 = spare(zero band).
DVE_CH2K = {2: 6, 4: 7, 6: 8, 8: 9, 10: 10, 12: 11, 14: 12}
ACT_CH2K = {3: 2, 7: 3, 11: 4, 15: 5}
POOL_CH2K = {5: 13, 9: 14, 13: 15}
Z_CH, SPARE_CH = 0, 1
# band DMA split by pair groups (channel ranges), in arrival order; the
# first two ride SP's HWDGE, the tiny last group rides Pool's SWDGE so its
# (+900ns) completion sem gates only the final pair's two passes
BAND_SPLITS = [(0, 6), (6, 12), (12, 16)]

N_PE_WARMUP = 3


def emit_pre_tile(nc, out_d):
    """Raw fin tensor allocated outside the tile pools (address fixed at
    emission); the out DMA itself is a plain HWDGE dma_start in-tile."""
    fin_t = nc.alloc_sbuf_tensor("fin_raw", [128, 1, 1, 16], f32)
    return out_d, fin_t


def emit_kernel(nc, tc, ctx, xt_d, bnd_ds, cst_d, pre):
    sb = ctx.enter_context(tc.tile_pool(name="sb", bufs=1))
    psp = ctx.enter_context(tc.tile_pool(name="psp", bufs=1, space=bass.MemorySpace.PSUM))
    DR = mybir.MatmulPerfMode.DoubleRow
    out_d, fin_t = pre
    fin = fin_t.ap()

    # ---- input DMAs ------------------------------------------------------
    # HWDGE triggers on SP in program order: x first (critical), then band
    # groups in pair order. consts ride Pool's SWDGE (engine idle early).
    xt = sb.tile([T, B, F], f32, tag="xt")
    nc.sync.dma_start(out=xt[:], in_=xt_d)
    xt2 = xt[:].rearrange("u b f -> u (b f)")  # [128, 320]

    sla = sb.tile([128, 2, Q, 128], f8, tag="sla")  # [u, bank, ch, d']
    for (c0, c1), bd in zip(BAND_SPLITS, bnd_ds):
        nc.sync.dma_start(out=sla[:, :, c0:c1, :].rearrange("p m c d -> p m (c d)"),
                          in_=bd)

    cst = sb.tile([128, 1216], f8, tag="cst")
    nc.gpsimd.dma_start(out=cst[:], in_=cst_d)
    triv = cst[:, 0:256].rearrange("p (j m) -> p j m", j=2)       # [128, 2, 128]
    idrv = cst[:, 256:1216].rearrange("p (j bf) -> p j bf", j=3)  # [128, 3, 320]

    # ---- early constant setup (engines idle until x lands) ---------------
    bia = sb.tile([128, 8], f32, tag="bia")
    for i, k in enumerate(ACT_CH2K.values()):
        nc.vector.memset(bia[:, i:i + 1], EPS - float(k))
    nc.vector.memset(bia[:, 5:6], 0.0)

    # pre-load the ACT Sign function table while waiting for x
    scr = sb.tile([128, 1], f32, tag="scr")
    nc.scalar.activation(out=scr[:], in_=bia[:, 5:6], func=AF.Sign,
                         bias=bia[:, 5:6], scale=1.0)

    oha = sb.tile([T, Q, BF], f8, tag="oha")
    nc.vector.memset(oha[:, SPARE_CH, :], 0.0)

    dw = sb.tile([128, 64], f8, tag="dw")
    nc.vector.memset(dw[:], 0.0)
    psD = psp.tile([64, 64], f32, tag="psD")
    for _ in range(N_PE_WARMUP):
        nc.tensor.matmul(psD[:], dw[:], dw[:], start=True, stop=True)

    # ---- masks -----------------------------------------------------------
    nc.vector.tensor_single_scalar(out=oha[:, Z_CH, :], in_=xt2, scalar=0.0,
                                   op=AL.is_equal)
    for ch, k in DVE_CH2K.items():
        nc.vector.tensor_single_scalar(out=oha[:, ch, :], in_=xt2,
                                       scalar=float(k) / 16.0, op=AL.is_ge)
    for ch, k in POOL_CH2K.items():
        nc.gpsimd.tensor_single_scalar(out=oha[:, ch, :], in_=xt2,
                                       scalar=float(k) / 16.0, op=AL.is_ge)
    for i, (ch, k) in enumerate(ACT_CH2K.items()):
        nc.scalar.activation(out=oha[:, ch, :], in_=xt2, func=AF.Sign,
                             bias=bia[:, i:i + 1], scale=16.0)

    # ---- matmul chains ---------------------------------------------------
    pA = psp.tile([128, BF], f32, tag="accA")
    pB = psp.tile([128, BF], f32, tag="accB")
    nc.tensor.matmul(pA[:], triv, idrv[:, 0:2], start=True, stop=False, perf_mode=DR)
    nc.tensor.matmul(pB[:], triv, idrv[:, 1:3], start=True, stop=False, perf_mode=DR)
    for ci in range(8):
        ca, cb = 2 * ci, 2 * ci + 1
        last = ci == 7
        nc.tensor.matmul(pA[:], sla[:, 0, ca:cb + 1, :], oha[:, ca:cb + 1, :],
                         start=False, stop=last, perf_mode=DR)
        nc.tensor.matmul(pB[:], sla[:, 1, ca:cb + 1, :], oha[:, ca:cb + 1, :],
                         start=False, stop=last, perf_mode=DR)

    # ---- parity + grouped reduce + threshold -----------------------------
    # i16 throughout: 2-byte packed operands unlock DVE 2x/4x modes; values
    # fit (S <= 256, group sums <= 40)
    si = sb.tile([128, 2, BF], i16, tag="si")
    nc.vector.tensor_copy(out=si[:, 0], in_=pA[:])
    nc.scalar.activation(out=si[:, 1], in_=pB[:], func=AF.Copy, bias=0.0, scale=1.0)
    par = sb.tile([128, 2, B, F], i16, tag="par")
    nc.vector.tensor_single_scalar(out=par[:, 0].rearrange("p b f -> p (b f)"),
                                   in_=si[:, 0], scalar=1, op=AL.bitwise_and)
    nc.vector.tensor_single_scalar(out=par[:, 1].rearrange("p b f -> p (b f)"),
                                   in_=si[:, 1], scalar=1, op=AL.bitwise_and)
    red = sb.tile([128, 2, B], i16, tag="red")
    with nc.allow_low_precision(reason="exact small-int accumulation (<=40)"):
        nc.vector.tensor_reduce(out=red[:], in_=par[:],
                                axis=mybir.AxisListType.X, op=AL.add)
    # threshold on DVE, one op: device classifies to {0, 2}; the constant
    # -1 relabel to {-1, +1} happens during host-side unshard/assembly
    nc.vector.tensor_scalar(out=fin[:, 0, 0, :],
                            in0=red[:].rearrange("p m b -> p (m b)"),
                            scalar1=20, scalar2=2.0, op0=AL.is_gt, op1=AL.mult)
    nc.sync.dma_start(out=out_d, in_=fin)


def build_nc():
    nc = bacc.Bacc("TRN2", target_bir_lowering=False, debug=False)
    # Startup-barrier surgery. The Bass-constructor barrier only orders the
    # const-AP registration memsets (which nothing in this kernel reads, and
    # which are moved to DVE where they are free). SP's only pre-compute work
    # is firing the input DMA triggers, so release SP from the barrier: drop
    # its waits and its release-decrement, and lower Pool's release-add from
    # 4 to 3 so the gather/release accounting still balances for the other
    # engines (final sem state unchanged; no negative-sem transitions).
    _ms_n = 0
    for bb in nc.m.functions[0].blocks:
        for ins in bb.instructions:
            si = ins.sync_info
            if type(ins).__name__ == "InstMemset" and ins.engine == mybir.EngineType.Pool:
                ins.engine = mybir.EngineType.DVE
            if not si:
                continue
            if any("barrier" in str(w) for w in si.on_wait):
                si.on_wait = [w for w in si.on_wait if "barrier" not in str(w)]
            if any("barrier" in str(u) for u in si.on_update):
                si.on_update = [u for u in si.on_update if "barrier" not in str(u)]
        break
    xt_d = nc.dram_tensor("xt", [T, B, F], f32, kind="ExternalInput")
    bnd_ds = [nc.dram_tensor(f"bnd{i}", [128, 2 * (c1 - c0) * 128], f8,
                             kind="ExternalInput")
              for i, (c0, c1) in enumerate(BAND_SPLITS)]
    cst_d = nc.dram_tensor("cst", [128, 1216], f8, kind="ExternalInput")
    out_d = nc.dram_tensor("out", [1, 128, 1, 16], f32, kind="ExternalOutput")
    pre = emit_pre_tile(nc, out_d[:])
    with tile.TileContext(nc) as tc:
        with ExitStack() as ctx:
            emit_kernel(nc, tc, ctx, xt_d[:], [bd[:] for bd in bnd_ds],
                        cst_d[:], pre)
    # Closing-barrier surgery: ACT/PE/DVE have no DMA-ring duties and their
    # results are all consumed via Tile data-flow sems, so they may pass the
    # closing barriers and halt without waiting. They keep their gather
    # increments (Pool still waits for all four engines -> its DGE drain
    # stays ordered after SP's out-DMA wait); their release decrements are
    # removed and Pool's release add drops 4 -> 1 so only SP's handshake
    # remains and no semaphore goes negative.
    EARLY = (mybir.EngineType.Activation, mybir.EngineType.PE, mybir.EngineType.DVE)
    seen = {}
    for bb in list(nc.m.functions[0].blocks)[1:]:
        for ins in bb.instructions:
            si = ins.sync_info
            if not si:
                continue
            is_bar = (any("barrier" in str(w) for w in si.on_wait)
                      or any("barrier" in str(u) for u in si.on_update))
            if not is_bar:
                continue
            n = seen.get(ins.engine, 0)
            seen[ins.engine] = n + 1
            if n >= 2:
                # second closing barrier: redundant once the first orders
                # Pool's drain after SP -> strip entirely for all engines
                si.on_wait = [w for w in si.on_wait if "barrier" not in str(w)]
                si.on_update = [u for u in si.on_update if "barrier" not in str(u)]
                continue
            if ins.engine in EARLY:
                si.on_wait = [w for w in si.on_wait if "barrier" not in str(w)]
                si.on_update = [u for u in si.on_update
                                if not ("release" in str(u) and "sem-dec" in str(u))]
            for u in si.on_update:
                if ("release" in str(u) and "sem-add-imm" in str(u)
                        and u.update_value == 4):
                    u.update_value = 1
    # Final-block handshake: Pool's closing drains only need to follow the
    # output DMA. Point Pool's gather-wait directly at the out-DMA completion
    # sem (same one SP waits) and delete the SP<->Pool release ping-pong, so
    # both engines drain in parallel right after the DMA lands.
    blocks = list(nc.m.functions[0].blocks)
    dma_w = None
    for bb in blocks:
        for ins in bb.instructions:
            si = ins.sync_info
            if si and ins.engine == mybir.EngineType.SP:
                for w in si.on_wait:
                    if "DMAHW4" in str(w):
                        dma_w = w
    if dma_w is not None:
        for ins in blocks[-1].instructions:
            si = ins.sync_info
            if not si:
                continue
            if ins.engine == mybir.EngineType.Pool and any(
                    "gather" in str(w) for w in si.on_wait):
                si.on_wait = [dma_w]
                si.on_update = [u for u in si.on_update if "gather" not in str(u)]
            elif ins.engine == mybir.EngineType.Pool:
                si.on_update = [u for u in si.on_update if "release" not in str(u)]
            if ins.engine == mybir.EngineType.SP:
                si.on_wait = [w for w in si.on_wait if "release" not in str(w)]
                si.on_update = [u for u in si.on_update if "release" not in str(u)]
    nc.compile()
    return nc


def make_in_maps(x, level_hvs, id_hvs):
    x = np.asarray(x, dtype=np.float32)
    L = np.asarray(level_hvs, dtype=np.int32)
    ID = np.asarray(id_hvs, dtype=np.int32)
    # time-reverse + transpose to [T, B, F] (so band indices are u + d')
    xt = np.ascontiguousarray(x[:, ::-1, :].transpose(1, 0, 2))

    # signed delta band tables per channel
    Btab = np.zeros((Q, D), np.float32)
    for ch, k in {**DVE_CH2K, **ACT_CH2K, **POOL_CH2K}.items():
        Btab[ch] = (L[k - 1] - L[k - 2]).astype(np.float32)
        if ch in ACT_CH2K:
            Btab[ch] *= 0.5  # +-1 sign-masks contribute h*Delta/2
    Btab[Z_CH] = (L[15] - L[0]).astype(np.float32)
    # Btab[SPARE_CH] stays 0
    Btab2 = np.ascontiguousarray(
        np.concatenate([Btab, Btab], axis=1)).astype(ml_dtypes.float8_e4m3)

    # constant id pass table: id + L0 + sum_{k in ACT} Delta_k/2 = id+L0/2+L4/2
    ks = sorted(ACT_CH2K.values())
    assert ks == list(range(ks[0], ks[0] + len(ks)))
    idp = ID.astype(np.float32) + 0.5 * L[ks[0] - 2] + 0.5 * L[ks[-1] - 1]
    idp2 = np.concatenate([idp, idp], axis=1)  # [40, 4096]

    m = np.arange(128)
    tri = np.zeros((128, 2, 128), np.float32)
    tri[:, 0, :] = (m[None, :] < m[:, None])
    tri[:, 1, :] = (m[None, :] >= m[:, None])
    tri8 = tri.reshape(128, 256).astype(ml_dtypes.float8_e4m3)

    flat = Btab2.reshape(-1)
    in_maps = []
    for c in range(NCORE):
        d0 = c * DS
        s = (d0 - 127) % D
        band = np.lib.stride_tricks.as_strided(
            flat[s:], shape=(128, Q, DS), strides=(1, 2 * D, 1))
        # [u, bank, ch, d']
        bnd = np.ascontiguousarray(
            np.asarray(band).reshape(128, Q, 2, 128).transpose(0, 2, 1, 3))
        core_map = {"xt": xt, }
        for i, (c0, c1) in enumerate(BAND_SPLITS):
            core_map[f"bnd{i}"] = np.ascontiguousarray(
                bnd[:, :, c0:c1, :]).reshape(128, 2 * (c1 - c0) * 128)
        s2 = (d0 - 128) % D
        idt_c = idp2[:, s2:s2 + 384].T                         # [384, 40]
        idt_full = np.broadcast_to(idt_c[:, None, :], (384, B, F)).reshape(384, BF)
        idt_r = np.ascontiguousarray(
            idt_full.reshape(3, 128, BF).transpose(1, 0, 2)).reshape(128, 3 * BF)
        core_map["cst"] = np.ascontiguousarray(np.concatenate(
            [tri8, idt_r.astype(ml_dtypes.float8_e4m3)], axis=1))
        in_maps.append(core_map)
    return in_maps


_NC_CACHE = {}


def kernel(x, level_hvs, id_hvs):
    if "nc" not in _NC_CACHE:
        _NC_CACHE["nc"] = build_nc()
    nc = _NC_CACHE["nc"]
    in_maps = make_in_maps(x, level_hvs, id_hvs)
    res = run_bass_kernel_spmd(nc, in_maps, list(range(NCORE)))
    full = np.empty((B, D), dtype=np.float32)
    for c in range(NCORE):
        o = np.asarray(res.results[c]["out"]).reshape(128, 2, B)  # [p, mc, b]
        full[:, c * DS:(c + 1) * DS] = o.transpose(2, 1, 0).reshape(B, DS) - 1.0
    return full
